# revision 1
# baseline (speedup 1.0000x reference)
"""Trainium2 Bass kernel for nn_AttentionLayer_45629732552708.

reference:
    scores  = tanh(q @ k + b)          # [B, TQ, TK], b broadcast over keys
    weights = softmax(scores, axis=-1)
    out     = weights @ v              # [B, TQ, DV]

Shapes (fp32): q [8, 2048, 1024], k [8, 1024, 2048], v [8, 2048, 1024],
b [2048].  Sharding: data-parallel over batch, one batch element per
NeuronCore (8 cores).

Per-core algorithm (no max-subtraction needed: tanh bounds scores to
[-1, 1], so exp is always in [e^-1, e]):
  Phase A: S^T[ki] = (q @ k)^T computed as k-tile-stationary matmuls so
           keys land on the partition axis; bias b is then a per-partition
           ACT bias.  P^T[ki] = exp(tanh(S^T + b)) stored fp16.
  Phase B: out[qa]  = sum_ki P^T[ki,qa].T @ v[ki]   (PSUM accumulation)
           den[qa]  = sum_ki P^T[ki,qa].T @ ones
           out      = out * reciprocal(den)         (DVE)
fp16 matmuls (1 cycle/row on PE vs 4 for fp32); fp32->fp16 casts are done
inside SWDGE DMA loads; q^T is produced by DMA x-bar transposes.
"""

import numpy as np

import concourse.bass as bass
import concourse.mybir as mybir
import concourse.tile as tile
from concourse import bacc
from concourse import bass_utils

F32 = mybir.dt.float32
F16 = mybir.dt.float16
AF = mybir.ActivationFunctionType

B, TQ, TK, D, DV = 8, 2048, 2048, 1024, 1024
P = 128
NKI = TK // P   # 16 key tiles
ND = D // P     # 8 contraction chunks
NQA = TQ // P   # 16 query tiles
N_CORES = 8


def _emit(tc, nc, q_d, k_d, v_d, b_d, o_d):
    with (
        tc.tile_pool(name="persist", bufs=1) as persist,
        tc.tile_pool(name="qstage_p", bufs=1) as qstage_p,
        tc.tile_pool(name="scratch", bufs=1) as scratch,
        tc.tile_pool(name="psum", bufs=1, space="PSUM") as psum_pool,
    ):
        # --- constants / small tiles ---
        ones16 = persist.tile([P, 1], F16, name="ones16")
        nc.vector.memset(ones16[:], 1.0)
        b_sb = persist.tile([P, NKI], F32, name="b_sb")
        nc.sync.dma_start(b_sb[:], b_d[:, :])

        # --- k: cast-load fp32 -> fp16, [128 d, 2048 k] per d-chunk ---
        k16 = []
        for d in range(ND):
            kt = persist.tile([P, TK], F16, name=f"k16_{d}")
            nc.gpsimd.dma_start(kt[:], k_d[d * P:(d + 1) * P, :])
            k16.append(kt)

        # --- v: cast-load fp32 -> fp16, [128 k, 1024 v] per k-tile ---
        v16 = []
        for ki in range(NKI):
            vt = persist.tile([P, DV], F16, name=f"v16_{ki}")
            nc.gpsimd.dma_start(vt[:], v_d[ki * P:(ki + 1) * P, :])
            v16.append(vt)

        # --- q: cast-load then DMA x-bar transpose to q^T ---
        # qT[qc] is [128 d_lo, 8 d_hi, 512 q]: partition = d within chunk.
        qT = []
        for qc in range(4):
            qt = persist.tile([P, ND, 512], F16, name=f"qT_{qc}")
            qT.append(qt)
        for qi in range(NQA):
            q16 = qstage_p.tile([P, D], F16, name="q16", tag="q16", bufs=4)
            nc.gpsimd.dma_start(q16[:], q_d[qi * P:(qi + 1) * P, :])
            qc, qs = divmod(qi, 4)
            nc.sync.dma_start(
                qT[qc][:, :, qs * P:(qs + 1) * P], q16[:], transpose=True
            )

        # --- P^T tiles: [128 k, 2048 q] fp16 per key tile ---
        p16 = []
        for ki in range(NKI):
            pt = persist.tile([P, TQ], F16, name=f"p16_{ki}")
            p16.append(pt)

        # --- Phase A: S^T = (q@k)^T, P^T = exp(tanh(S^T + b)) ---
        for ki in range(NKI):
            for qh in range(2):
                s_ps = psum_pool.tile(
                    [P, 1024], F32, name="acc", tag="acc", bufs=3
                )
                for d in range(ND):
                    lhsT = k16[d][:, ki * P:(ki + 1) * P]
                    for qc in range(2):
                        nc.tensor.matmul(
                            s_ps[:, qc * 512:(qc + 1) * 512],
                            lhsT,
                            qT[2 * qh + qc][:, d, :],
                            start=(d == 0),
                            stop=(d == ND - 1),
                        )
                t_sb = scratch.tile(
                    [P, 1024], F32, name="t_sb", tag="t_sb", bufs=3
                )
                nc.scalar.activation(
                    t_sb[:], s_ps[:], AF.Tanh, bias=b_sb[:, ki:ki + 1]
                )
                nc.scalar.activation(
                    p16[ki][:, qh * 1024:(qh + 1) * 1024], t_sb[:], AF.Exp
                )

        # --- Phase B: out = P^T.T @ v, den = P^T.T @ 1, normalize ---
        for qa in range(NQA):
            o_ps = psum_pool.tile([P, 1024], F32, name="acc", tag="acc", bufs=3)
            den_ps = psum_pool.tile([P, 1], F32, name="den", tag="den", bufs=2)
            for ki in range(NKI):
                lhsT = p16[ki][:, qa * P:(qa + 1) * P]
                nc.tensor.matmul(
                    o_ps[:, 0:512], lhsT, v16[ki][:, 0:512],
                    start=(ki == 0), stop=(ki == NKI - 1),
                )
                nc.tensor.matmul(
                    o_ps[:, 512:1024], lhsT, v16[ki][:, 512:1024],
                    start=(ki == 0), stop=(ki == NKI - 1),
                )
                nc.tensor.matmul(
                    den_ps[:], lhsT, ones16[:],
                    start=(ki == 0), stop=(ki == NKI - 1),
                )
            recip = scratch.tile([P, 1], F32, name="recip", tag="recip", bufs=2)
            nc.vector.reciprocal(recip[:], den_ps[:])
            o_sb = scratch.tile([P, 1024], F32, name="o_sb", tag="o_sb", bufs=2)
            nc.vector.tensor_scalar_mul(o_sb[:], o_ps[:], recip[:])
            nc.sync.dma_start(o_d[qa * P:(qa + 1) * P, :], o_sb[:])


def build_module():
    nc = bacc.Bacc(None, target_bir_lowering=False, debug=False)
    with tile.TileContext(nc) as tc:
        with tc.tile_pool(name="dram", bufs=1, space="DRAM") as dram:
            q_d = dram.tile([TQ, D], F32, kind="ExternalInput",
                            name="q_in", uniquify=False)
            k_d = dram.tile([D, TK], F32, kind="ExternalInput",
                            name="k_in", uniquify=False)
            v_d = dram.tile([TK, DV], F32, kind="ExternalInput",
                            name="v_in", uniquify=False)
            b_d = dram.tile([P, NKI], F32, kind="ExternalInput",
                            name="b_in", uniquify=False)
            o_d = dram.tile([TQ, DV], F32, kind="ExternalOutput",
                            name="o_out", uniquify=False)
            _emit(tc, nc, q_d[:], k_d[:], v_d[:], b_d[:], o_d[:])
    nc.compile()
    return nc


_MODULE = None


def _get_module():
    global _MODULE
    if _MODULE is None:
        _MODULE = build_module()
    return _MODULE


def make_in_maps(q, k, v, b):
    # b rearranged host-side to [128, 16]: b_pk[p, j] = b[j*128 + p]
    b_pk = np.ascontiguousarray(b.reshape(NKI, P).T).astype(np.float32)
    in_maps = []
    for i in range(N_CORES):
        in_maps.append({
            "q_in": np.ascontiguousarray(q[i], dtype=np.float32),
            "k_in": np.ascontiguousarray(k[i], dtype=np.float32),
            "v_in": np.ascontiguousarray(v[i], dtype=np.float32),
            "b_in": b_pk,
        })
    return in_maps


def run(q, k, v, b, trace=False):
    """Run on hardware; returns (output [8, 2048, 1024] f32, BassKernelResults)."""
    nc = _get_module()
    in_maps = make_in_maps(q, k, v, b)
    res = bass_utils.run_bass_kernel_spmd(
        nc, in_maps, core_ids=list(range(N_CORES)), trace=trace
    )
    out = np.stack([r["o_out"] for r in res.results], axis=0).astype(np.float32)
    return out, res


def kernel(q, k, v, b):
    out, _ = run(np.asarray(q), np.asarray(k), np.asarray(v), np.asarray(b))
    return out
